# revision 1
# baseline (speedup 1.0000x reference)
"""2-layer GAT (heads=1) + linear classifier, distributed over 8 TRN2 NeuronCores.

Strategy (graph/data parallel):
  - Nodes (and their incident edges, by destination) are sharded across the 8
    cores: core c owns dst nodes [c*1250, (c+1)*1250).
  - Small weights are replicated; per-layer source features are exchanged with
    an AllGather of the fp16 feature table [h | h@a_src].
  - Per core, edges are sorted by dst and processed in chunks of 128 on the
    partition axis: indirect-DMA row gather of h[src], attention scalars on
    the ACT engine, and the weighted scatter-sum is a PE matmul against a
    one-hot selection matrix S[e, d] = exp(e_edge) * (dst_local[e] == d).
  - ad[dst] per edge comes from a tiny matmul against a host-known staircase
    0/1 matrix S0T[d, e] (edges sorted by dst => contiguous runs).
"""
import sys

sys.path.insert(0, "/opt/trn_rl_repo")

import numpy as np

N_NODES = 10000
N_EDGES = 160000
F_IN, D, N_CLS = 300, 1024, 10
NEG_SLOPE = 0.2
NC_ = 8                      # cores
NPC = N_NODES // NC_         # real nodes per core (1250)
PADN = 1280                  # padded nodes per core (10 * 128)
NB = 10                      # dst blocks of 128 per core
T = D + 2                    # table row: [h(1024) | as | pad]
FKX = 3                      # 384 / 128 k-chunks for layer 1
DKX = D // 128               # 8 k-chunks for layer 2
F_PAD = FKX * 128            # 384

_CACHE = {}


# ----------------------------------------------------------------- host prep
def _row_of_node(n):
    # table row of global node id (per-core stripes padded to PADN rows)
    return (n // NPC) * PADN + (n % NPC)


def _edge_metadata(src, dst, k_blk=None):
    """Per-core chunked edge schedule. Returns (K_BLK, per-core dict arrays)."""
    order_all = []
    counts = []
    for c in range(NC_):
        sel = (dst >= c * NPC) & (dst < (c + 1) * NPC)
        s_c = src[sel]
        d_c = dst[sel] - c * NPC
        o = np.argsort(d_c, kind="stable")
        s_c, d_c = s_c[o], d_c[o]
        order_all.append((s_c, d_c))
        for b in range(NB):
            lo = np.searchsorted(d_c, b * 128, side="left")
            hi = np.searchsorted(d_c, (b + 1) * 128, side="left")
            counts.append(hi - lo)
    K_BLK = int(np.ceil(max(counts) / 128))
    if k_blk is not None:
        K_BLK = max(K_BLK, k_blk)
    EBP = K_BLK * 128
    NCH = NB * K_BLK
    metas = []
    for c in range(NC_):
        s_c, d_c = order_all[c]
        idx = np.zeros((128, NCH), np.int32)
        dstl = np.full((128, NCH), -1.0, np.float32)
        lo_a = np.zeros((128, NB), np.float32)
        hi_a = np.zeros((128, NB), np.float32)
        for b in range(NB):
            bs = np.searchsorted(d_c, b * 128, side="left")
            be = np.searchsorted(d_c, (b + 1) * 128, side="left")
            cnt = be - bs
            rows = _row_of_node(s_c[bs:be]).astype(np.int32)
            dl = (d_c[bs:be] - b * 128).astype(np.float32)
            flat_i = np.zeros(EBP, np.int32)
            flat_d = np.full(EBP, -1.0, np.float32)
            flat_i[:cnt] = rows
            flat_d[:cnt] = dl
            idx[:, b * K_BLK:(b + 1) * K_BLK] = flat_i.reshape(K_BLK, 128).T
            dstl[:, b * K_BLK:(b + 1) * K_BLK] = flat_d.reshape(K_BLK, 128).T
            dloc = d_c[bs:be] - b * 128
            nreal = min(128, NPC - b * 128)
            for dd in range(nreal):
                lo_a[dd, b] = np.searchsorted(dloc, dd, side="left")
                hi_a[dd, b] = np.searchsorted(dloc, dd, side="right")
        metas.append(dict(idx=idx, dstl=dstl, lo=lo_a, hi=hi_a))
    return K_BLK, metas


def _pack_inputs(x, edge_index, W1, att_src1, att_dst1, b1,
                 W2, att_src2, att_dst2, b2, fc_w, fc_b, k_blk=None):
    f16, f32 = np.float16, np.float32
    src = np.concatenate([edge_index[0], np.arange(N_NODES)]).astype(np.int64)
    dst = np.concatenate([edge_index[1], np.arange(N_NODES)]).astype(np.int64)
    K_BLK, metas = _edge_metadata(src, dst, k_blk)

    # per-feature-major packs with 128-partition folding
    def fold_k(a, kx):  # [kx*128, m] -> [128, kx*m]
        kxp, m = a.shape
        return np.ascontiguousarray(
            a.reshape(kx, 128, m).transpose(1, 0, 2).reshape(128, kx * m))

    W1p = np.zeros((F_PAD, D), f32)
    W1p[:F_IN] = W1
    w1sd = np.stack([W1p @ att_src1, W1p @ att_dst1], axis=1)  # [384, 2]
    w2sd = np.stack([W2 @ att_src2, W2 @ att_dst2], axis=1)    # [1024, 2]
    W1f = fold_k(W1p.astype(f16), FKX)                      # [128, 3*1024]
    W2f = fold_k(W2.astype(f16), DKX)                       # [128, 8*1024]
    FCf = fold_k(fc_w.astype(f16), DKX)                     # [128, 8*10]
    w1sdf = fold_k(w1sd.astype(f16), FKX)                   # [128, 3*2]
    w2sdf = fold_k(w2sd.astype(f16), DKX)                   # [128, 8*2]
    bb1 = np.broadcast_to(b1.astype(f32), (128, D)).copy()
    bb2 = np.broadcast_to(b2.astype(f32), (128, D)).copy()
    fcb = np.broadcast_to(fc_b.astype(f32), (128, N_CLS)).copy()

    in_maps = []
    for c in range(NC_):
        xs = np.zeros((PADN, F_PAD), f32)
        xs[:NPC, :F_IN] = x[c * NPC:(c + 1) * NPC]
        xTf = fold_k(np.ascontiguousarray(xs.T).astype(f16), FKX)  # [128, 3*1280]
        m = metas[c]
        in_maps.append({
            "xT": xTf, "W1": W1f, "W2": W2f, "FC": FCf,
            "w1sd": w1sdf, "w2sd": w2sdf,
            "bb1": bb1, "bb2": bb2, "fcb": fcb,
            "eidx": m["idx"], "dstl": m["dstl"], "lo": m["lo"], "hi": m["hi"],
        })
    return K_BLK, in_maps


# ------------------------------------------------------------- device program
def _build_program(K_BLK, reps=1, debug=False):
    import concourse.bacc as bacc
    import concourse.bass as bass
    import concourse.mybir as mybir
    from concourse.tile import TileContext

    f32, f16, i32 = mybir.dt.float32, mybir.dt.float16, mybir.dt.int32
    AF = mybir.ActivationFunctionType
    ALU = mybir.AluOpType
    AX = mybir.AxisListType
    EBP = K_BLK * 128
    NCH = NB * K_BLK

    nc = bacc.Bacc("TRN2", target_bir_lowering=False, debug=False, num_devices=NC_)
    d_xT = nc.dram_tensor("xT", [128, FKX * PADN], f16, kind="ExternalInput")
    d_W1 = nc.dram_tensor("W1", [128, FKX * D], f16, kind="ExternalInput")
    d_W2 = nc.dram_tensor("W2", [128, DKX * D], f16, kind="ExternalInput")
    d_FC = nc.dram_tensor("FC", [128, DKX * N_CLS], f16, kind="ExternalInput")
    d_w1sd = nc.dram_tensor("w1sd", [128, FKX * 2], f16, kind="ExternalInput")
    d_w2sd = nc.dram_tensor("w2sd", [128, DKX * 2], f16, kind="ExternalInput")
    d_bb1 = nc.dram_tensor("bb1", [128, D], f32, kind="ExternalInput")
    d_bb2 = nc.dram_tensor("bb2", [128, D], f32, kind="ExternalInput")
    d_fcb = nc.dram_tensor("fcb", [128, N_CLS], f32, kind="ExternalInput")
    d_idx = nc.dram_tensor("eidx", [128, NCH], i32, kind="ExternalInput")
    d_dstl = nc.dram_tensor("dstl", [128, NCH], f32, kind="ExternalInput")
    d_lo = nc.dram_tensor("lo", [128, NB], f32, kind="ExternalInput")
    d_hi = nc.dram_tensor("hi", [128, NB], f32, kind="ExternalInput")
    d_out = nc.dram_tensor("out", [PADN, N_CLS], f32, kind="ExternalOutput")

    shard = [nc.dram_tensor(f"shard{l}", [PADN, T], f16) for l in (1, 2)]
    table = [nc.dram_tensor(f"table{l}", [NC_ * PADN, T], f16, addr_space="Shared")
             for l in (1, 2)]
    dbg = {}
    if debug:
        dbg["sh1"] = nc.dram_tensor("dbg_sh1", [PADN, T], f16, kind="ExternalOutput")
        dbg["tab1"] = nc.dram_tensor("dbg_tab1", [128, T], f16, kind="ExternalOutput")
        dbg["adc1"] = nc.dram_tensor("dbg_adc1", [128, NB], f16, kind="ExternalOutput")
        dbg["embT1"] = nc.dram_tensor("dbg_embT1", [128, PADN], f16,
                                      kind="ExternalOutput")
        dbg["sh2"] = nc.dram_tensor("dbg_sh2", [PADN, T], f16, kind="ExternalOutput")
        dbg["embT2"] = nc.dram_tensor("dbg_embT2", [128, PADN], f16,
                                      kind="ExternalOutput")

    with TileContext(nc) as tc:
        with (
            tc.tile_pool(name="const", bufs=1) as cp,
            tc.tile_pool(name="embT", bufs=1) as ep,
            tc.tile_pool(name="work", bufs=2) as wp,
            tc.tile_pool(name="gath", bufs=6) as gp,
            tc.tile_pool(name="stair", bufs=1) as stp,
            tc.tile_pool(name="s0tp", bufs=2) as s0p,
            tc.tile_pool(name="smat", bufs=6) as sp,
            tc.tile_pool(name="ps", bufs=1, space="PSUM") as ps,
        ):
            # ---- constants
            xT = cp.tile([128, FKX * PADN], f16)
            nc.sync.dma_start(out=xT[:], in_=d_xT[:])
            W1 = cp.tile([128, FKX * D], f16)
            nc.sync.dma_start(out=W1[:], in_=d_W1[:])
            W2 = cp.tile([128, DKX * D], f16)
            nc.sync.dma_start(out=W2[:], in_=d_W2[:])
            FC = cp.tile([128, DKX * N_CLS], f16)
            nc.sync.dma_start(out=FC[:], in_=d_FC[:])
            wvec = {}
            for nm, dd in (("w1sd", d_w1sd), ("w2sd", d_w2sd)):
                tt = cp.tile([128, dd.shape[1]], f16, tag=nm)
                nc.sync.dma_start(out=tt[:], in_=dd[:])
                wvec[nm] = tt
            bb1 = cp.tile([128, D], f32)
            nc.sync.dma_start(out=bb1[:], in_=d_bb1[:])
            bb2 = cp.tile([128, D], f32)
            nc.sync.dma_start(out=bb2[:], in_=d_bb2[:])
            fcb = cp.tile([128, N_CLS], f32)
            nc.sync.dma_start(out=fcb[:], in_=d_fcb[:])
            idx = cp.tile([128, NCH], i32)
            nc.sync.dma_start(out=idx[:], in_=d_idx[:])
            dstl = cp.tile([128, NCH], f32)
            nc.sync.dma_start(out=dstl[:], in_=d_dstl[:])
            lo_t = cp.tile([128, NB], f32)
            nc.sync.dma_start(out=lo_t[:], in_=d_lo[:])
            hi_t = cp.tile([128, NB], f32)
            nc.sync.dma_start(out=hi_t[:], in_=d_hi[:])
            iota_e = cp.tile([128, EBP], f32)
            nc.gpsimd.iota(iota_e[:], pattern=[[1, EBP]], base=0,
                           channel_multiplier=0,
                           allow_small_or_imprecise_dtypes=True)
            iota_f = cp.tile([128, 128], f32)
            nc.vector.tensor_copy(out=iota_f[:], in_=iota_e[:, 0:128])
            ones16 = cp.tile([128, 1], f16)
            nc.gpsimd.memset(ones16[:], 1.0)
            ident = cp.tile([128, 128], f16)
            nc.gpsimd.memset(ident[:], 0.0)
            nc.gpsimd.affine_select(
                out=ident[:], in_=ident[:], compare_op=ALU.not_equal, fill=1.0,
                base=0, pattern=[[-1, 128]], channel_multiplier=1)

            embT = {(l, kc): ep.tile([128, PADN], f16, tag=f"embT{l}_{kc}",
                                     name=f"embT{l}_{kc}")
                    for l in (1, 2) for kc in range(DKX)}

            def linear_phase(layer):
                """h_l = in @ W_l (+ as/ad); write fp16 [h|as] shard rows."""
                kx = FKX if layer == 1 else DKX
                Wt = W1 if layer == 1 else W2
                wsd = wvec["w1sd"] if layer == 1 else wvec["w2sd"]
                adcol = cp.tile([128, NB], f16, tag=f"adcol{layer}")
                for mt in range(NB):
                    if layer == 1:
                        def lhs(kc):
                            return xT[:, kc * PADN + mt * 128: kc * PADN + (mt + 1) * 128]
                    else:
                        def lhs(kc):
                            return embT[(1, kc)][:, mt * 128:(mt + 1) * 128]
                    stage = wp.tile([128, T], f16, tag="stage")
                    for half in range(2):
                        hh = ps.tile([128, 512], f32, tag="acc", name="hh", bufs=4)
                        for kc in range(kx):
                            nc.tensor.matmul(
                                hh[:], lhsT=lhs(kc),
                                rhs=Wt[:, kc * D + half * 512: kc * D + half * 512 + 512],
                                start=(kc == 0), stop=(kc == kx - 1))
                        nc.vector.tensor_copy(
                            out=stage[:, half * 512:half * 512 + 512], in_=hh[:])
                    pasd = ps.tile([128, 2], f32, tag="psmall", name="pasd")
                    for kc in range(kx):
                        nc.tensor.matmul(pasd[:], lhsT=lhs(kc),
                                         rhs=wsd[:, 2 * kc:2 * kc + 2],
                                         start=(kc == 0), stop=(kc == kx - 1))
                    nc.vector.tensor_copy(out=stage[:, 1024:1025], in_=pasd[:, 0:1])
                    nc.vector.memset(stage[:, 1025:1026], 0.0)
                    nc.vector.tensor_copy(out=adcol[:, mt:mt + 1], in_=pasd[:, 1:2])
                    nc.sync.dma_start(
                        out=shard[layer - 1][mt * 128:(mt + 1) * 128, :], in_=stage[:])
                return adcol

            def edge_phase(layer, adcol):
                """softmax-weighted aggregation into emb; transpose into embT."""
                tab = table[layer - 1]
                bb = bb1 if layer == 1 else bb2
                for b in range(NB):
                    s0t = s0p.tile([128, EBP], f16, tag="s0t")
                    aA = stp.tile([128, EBP], f16, tag="stairA")
                    aB = stp.tile([128, EBP], f16, tag="stairB")
                    nc.vector.tensor_scalar(out=aA[:], in0=iota_e[:],
                                            scalar1=lo_t[:, b:b + 1], scalar2=None,
                                            op0=ALU.is_ge)
                    nc.vector.tensor_scalar(out=aB[:], in0=iota_e[:],
                                            scalar1=hi_t[:, b:b + 1], scalar2=None,
                                            op0=ALU.is_ge)
                    nc.vector.tensor_tensor(out=s0t[:], in0=aA[:], in1=aB[:],
                                            op=ALU.subtract)
                    padg = ps.tile([128, K_BLK], f32, tag="ptr", name="padg",
                                   bufs=2)
                    for k in range(K_BLK):
                        nc.tensor.matmul(padg[:, k:k + 1],
                                         lhsT=s0t[:, k * 128:(k + 1) * 128],
                                         rhs=adcol[:, b:b + 1], start=True, stop=True)
                    adg = wp.tile([128, K_BLK], f32, tag="adg")
                    nc.vector.tensor_copy(out=adg[:], in_=padg[:])
                    pn0 = ps.tile([128, 512], f32, tag="acc", name="pn0", bufs=4)
                    pn1 = ps.tile([128, 512], f32, tag="acc", name="pn1", bufs=4)
                    pde = ps.tile([128, 1], f32, tag="pde", name="pde")
                    for k in range(K_BLK):
                        c = b * K_BLK + k
                        m = gp.tile([128, T], f16, tag="m")
                        nc.gpsimd.indirect_dma_start(
                            out=m[:], out_offset=None, in_=tab[:],
                            in_offset=bass.IndirectOffsetOnAxis(
                                ap=idx[:, c:c + 1], axis=0))
                        ee = wp.tile([128, 1], f32, tag="ee")
                        nc.scalar.activation(ee[:], m[:, 1024:1025], AF.Prelu,
                                             bias=adg[:, k:k + 1], scale=1.0,
                                             alpha=NEG_SLOPE)
                        ex = wp.tile([128, 1], f32, tag="ex")
                        nc.scalar.activation(ex[:], ee[:], AF.Exp)
                        s = sp.tile([128, 128], f16, tag="s")
                        nc.vector.tensor_scalar(out=s[:], in0=iota_f[:],
                                                scalar1=dstl[:, c:c + 1],
                                                scalar2=ex[:, 0:1],
                                                op0=ALU.is_equal, op1=ALU.mult)
                        st, sp_ = (k == 0), (k == K_BLK - 1)
                        nc.tensor.matmul(pn0[:], lhsT=s[:], rhs=m[:, 0:512],
                                         start=st, stop=sp_)
                        nc.tensor.matmul(pn1[:], lhsT=s[:], rhs=m[:, 512:1024],
                                         start=st, stop=sp_)
                        nc.tensor.matmul(pde[:], lhsT=s[:], rhs=ones16[:],
                                         start=st, stop=sp_)
                    den = wp.tile([128, 1], f32, tag="den")
                    nc.vector.tensor_scalar(out=den[:], in0=pde[:], scalar1=1e-30,
                                            scalar2=None, op0=ALU.max)
                    rec = wp.tile([128, 1], f32, tag="rec")
                    nc.vector.reciprocal(out=rec[:], in_=den[:])
                    emb = wp.tile([128, D], f16, tag="emb")
                    for half in range(2):
                        pnh = pn0 if half == 0 else pn1
                        t1 = wp.tile([128, 512], f32, tag=f"t1_{half}")
                        nc.vector.tensor_scalar(out=t1[:], in0=pnh[:],
                                                scalar1=rec[:, 0:1], scalar2=None,
                                                op0=ALU.mult)
                        t2 = wp.tile([128, 512], f32, tag=f"t2_{half}")
                        nc.vector.tensor_tensor(
                            out=t2[:], in0=t1[:],
                            in1=bb[:, half * 512:half * 512 + 512], op=ALU.add)
                        nc.scalar.activation(emb[:, half * 512:half * 512 + 512],
                                             t2[:], AF.Relu)
                    for kc in range(DKX):
                        ptr = ps.tile([128, 128], f16, tag="ptr", name="ptr",
                                      bufs=2)
                        nc.tensor.transpose(ptr[:], in_=emb[:, kc * 128:(kc + 1) * 128],
                                            identity=ident[:])
                        nc.vector.tensor_copy(
                            out=embT[(layer, kc)][:, b * 128:(b + 1) * 128], in_=ptr[:])

            for _ in range(reps):
                adcol1 = linear_phase(1)
                nc.gpsimd.collective_compute(
                    "AllGather", mybir.AluOpType.bypass,
                    replica_groups=[list(range(NC_))],
                    ins=[shard[0][:]], outs=[table[0][:]])
                edge_phase(1, adcol1)
                adcol2 = linear_phase(2)
                nc.gpsimd.collective_compute(
                    "AllGather", mybir.AluOpType.bypass,
                    replica_groups=[list(range(NC_))],
                    ins=[shard[1][:]], outs=[table[1][:]])
                edge_phase(2, adcol2)
                if debug:
                    nc.gpsimd.dma_start(out=dbg["sh1"][:], in_=shard[0][:])
                    nc.gpsimd.dma_start(out=dbg["tab1"][:],
                                        in_=table[0][5000:5128, :])
                    ac = wp.tile([128, NB], f16, tag="dbgac")
                    nc.vector.tensor_copy(out=ac[:], in_=adcol1[:])
                    nc.sync.dma_start(out=dbg["adc1"][:], in_=ac[:])
                    nc.sync.dma_start(out=dbg["embT1"][:], in_=embT[(1, 0)][:])
                    nc.gpsimd.dma_start(out=dbg["sh2"][:], in_=shard[1][:])
                    nc.sync.dma_start(out=dbg["embT2"][:], in_=embT[(2, 0)][:])
                # classifier + row softmax
                for mt in range(NB):
                    pl = ps.tile([128, N_CLS], f32, tag="psmall", name="plog")
                    for kc in range(DKX):
                        nc.tensor.matmul(
                            pl[:], lhsT=embT[(2, kc)][:, mt * 128:(mt + 1) * 128],
                            rhs=FC[:, kc * N_CLS:(kc + 1) * N_CLS],
                            start=(kc == 0), stop=(kc == DKX - 1))
                    lg = wp.tile([128, N_CLS], f32, tag="lg")
                    nc.vector.tensor_tensor(out=lg[:], in0=pl[:], in1=fcb[:],
                                            op=ALU.add)
                    mx = wp.tile([128, 1], f32, tag="mx")
                    nc.vector.tensor_reduce(out=mx[:], in_=lg[:], axis=AX.X,
                                            op=ALU.max)
                    lgs = wp.tile([128, N_CLS], f32, tag="lgs")
                    nc.vector.tensor_scalar(out=lgs[:], in0=lg[:],
                                            scalar1=mx[:, 0:1], scalar2=None,
                                            op0=ALU.subtract)
                    eg = wp.tile([128, N_CLS], f32, tag="eg")
                    nc.scalar.activation(eg[:], lgs[:], AF.Exp)
                    sm = wp.tile([128, 1], f32, tag="sm")
                    nc.vector.tensor_reduce(out=sm[:], in_=eg[:], axis=AX.X,
                                            op=ALU.add)
                    rc = wp.tile([128, 1], f32, tag="rc")
                    nc.vector.reciprocal(out=rc[:], in_=sm[:])
                    ot = wp.tile([128, N_CLS], f32, tag="ot")
                    nc.vector.tensor_scalar(out=ot[:], in0=eg[:],
                                            scalar1=rc[:, 0:1], scalar2=None,
                                            op0=ALU.mult)
                    nc.sync.dma_start(out=d_out[mt * 128:(mt + 1) * 128, :],
                                      in_=ot[:])
    nc.compile()
    return nc


# ------------------------------------------------------------------ execution
class _Runner:
    """Cached-jit SPMD executor (axon/PJRT path of run_bass_kernel_spmd)."""

    def __init__(self, nc, n_cores=NC_):
        import jax
        from jax.sharding import Mesh, PartitionSpec
        from jax.experimental.shard_map import shard_map
        import concourse.mybir as mybir
        from concourse.bass2jax import (_bass_exec_p, install_neuronx_cc_hook,
                                        partition_id_tensor)

        install_neuronx_cc_hook()
        self.jax = jax
        self.n_cores = n_cores
        pname = nc.partition_id_tensor.name if nc.partition_id_tensor else None
        in_names, out_names, out_avals, zero_outs = [], [], [], []
        for alloc in nc.m.functions[0].allocations:
            if not isinstance(alloc, mybir.MemoryLocationSet):
                continue
            name = alloc.memorylocations[0].name
            if alloc.kind == "ExternalInput":
                if name != pname:
                    in_names.append(name)
            elif alloc.kind == "ExternalOutput":
                shape = tuple(alloc.tensor_shape)
                dtype = mybir.dt.np(alloc.dtype)
                out_names.append(name)
                out_avals.append(jax.core.ShapedArray(shape, dtype))
                zero_outs.append(np.zeros(shape, dtype))
        self.in_names, self.out_names = in_names, out_names
        self.out_avals, self.zero_outs = out_avals, zero_outs
        n_params = len(in_names)
        all_in = list(in_names) + list(out_names)
        if pname is not None:
            all_in.append(pname)

        def _body(*args):
            operands = list(args)
            if pname is not None:
                operands.append(partition_id_tensor())
            return tuple(_bass_exec_p.bind(
                *operands, out_avals=tuple(out_avals), in_names=tuple(all_in),
                out_names=tuple(out_names), lowering_input_output_aliases=(),
                sim_require_finite=True, sim_require_nnan=True, nc=nc))

        devices = jax.devices()[:n_cores]
        self.mesh = Mesh(np.asarray(devices), ("core",))
        n_outs = len(out_names)
        in_specs = (PartitionSpec("core"),) * (n_params + n_outs)
        out_specs = (PartitionSpec("core"),) * n_outs
        self.sharded = jax.jit(
            shard_map(_body, mesh=self.mesh, in_specs=in_specs,
                      out_specs=out_specs, check_rep=False),
            keep_unused=True)
        self.PartitionSpec = PartitionSpec

    def set_inputs(self, in_maps):
        jax = self.jax
        n = self.n_cores
        concat_in = [
            np.concatenate([np.asarray(in_maps[c][k]) for c in range(n)], axis=0)
            for k in self.in_names]
        concat_zero = [np.zeros((n * z.shape[0], *z.shape[1:]), z.dtype)
                       for z in self.zero_outs]
        sh = jax.sharding.NamedSharding(self.mesh, self.PartitionSpec("core"))
        self._args = [jax.device_put(a, sh) for a in concat_in + concat_zero]
        jax.block_until_ready(self._args)

    def run(self):
        return self.sharded(*self._args)

    def run_np(self):
        outs = self.jax.block_until_ready(self.run())
        n = self.n_cores
        return [
            {k: np.asarray(outs[i]).reshape(n, *self.out_avals[i].shape)[c]
             for i, k in enumerate(self.out_names)}
            for c in range(n)]


def _get_runner(K_BLK, reps=1, debug=False):
    key = (K_BLK, reps, debug)
    if key not in _CACHE:
        nc = _build_program(K_BLK, reps, debug)
        _CACHE[key] = _Runner(nc)
    return _CACHE[key]


def kernel(**inputs):
    inputs = {k: np.asarray(v) for k, v in inputs.items()}
    assert inputs["x"].shape == (N_NODES, F_IN)
    assert inputs["edge_index"].shape == (2, N_EDGES)
    K_BLK, in_maps = _pack_inputs(**inputs)
    runner = _get_runner(K_BLK)
    runner.set_inputs(in_maps)
    res = runner.run_np()
    out = np.concatenate([res[c]["out"][:NPC] for c in range(NC_)], axis=0)
    return out.astype(np.float32)

